# revision 6
# baseline (speedup 1.0000x reference)
"""Trainium2 Bass kernel for nn_Decoder_58377195487266.

Single-layer decoder: shared-head causal attention (d_k=32) + FFN(256->1024->256)
with two LayerNorms. B=16, T=2048, EMB=256.

Sharding: pure data-parallel over batch. 8 cores x 2 batches each, weights
replicated, no collectives.

Numerics/layout strategy (validated vs reference in numpy, absmax_rel ~6e-3):
  - fp8e4m3 (TRN FP8_EXP4) inputs for all heavy matmuls, with host-side scale
    folding (weights pre-scaled so values stay in e4m3's normal range; the
    inverse scales fold into free activation-scale slots downstream).
  - DoubleRow fp8 matmuls (2 k-tiles of 128 packed per instruction) for the
    q/k projections, the attention accumulate, and both FFN matmuls.
  - Scores matmul in bf16 (K=32) with 4-way PE row tiling (tile_position).
  - exp via ACT with the combined descale folded into the activation scale.
  - Softmax denominator via ones-column in v (paired/fp8), recip on DVE.
  - FFN residual (z2 = y1 + ff) accumulated into PSUM by a 16*I identity
    matmul (f32r), so the DVE pass is a cheap single-src tensor_scalar.
  - Batch phases ordered att(0),att(1),ffn(0),ffn(1) so the ACT table set
    switches exp<->gelu only twice per iteration.
"""

import math
import os

import numpy as np
import ml_dtypes

import concourse.bass as bass  # noqa: F401
import concourse.mybir as mybir
from concourse import bacc
from concourse.bass_utils import run_bass_kernel_spmd
from concourse.tile import TileContext

F32 = mybir.dt.float32
F32R = mybir.dt.float32r
BF16 = mybir.dt.bfloat16
F8 = mybir.dt.float8e4
I32 = mybir.dt.int32
AF = mybir.ActivationFunctionType
OP = mybir.AluOpType
DR = mybir.MatmulPerfMode.DoubleRow

B, T, EMB = 16, 2048, 256
DK = 32
HID = 4 * EMB
N_CORES = 8
B_LOC = B // N_CORES  # 2
NT = T // 128         # 16 t-blocks
NCH = T // 512        # 4 chunks
LN_EPS = 1e-5

# host-side scale folding (see module docstring)
SQ, SK, SV, SW1, SW2, SY = 64.0, 16.0, 16.0, 16.0, 16.0, 4.0
ESCALE = 1.0 / (SQ * SK * math.sqrt(DK))   # applied inside exp
GSCALE = 1.0 / (SY * SW1)                  # applied inside gelu
VPAD = 48                                  # v pair stride (DoubleRow needs %16==0)

RSQRT_MAGIC = 0x5F3759DF


def _nr_rsqrt(nc, pool, out, varp):
    """out = 1/sqrt(varp) via bit-trick + 3 Newton iterations, all on DVE."""
    sh = list(varp.shape)
    yi = pool.tile(sh, I32, tag="nr_i", bufs=2)
    magic = pool.tile(sh, I32, tag="nr_m", bufs=2)
    nc.vector.memset(magic[:], RSQRT_MAGIC)
    nc.vector.tensor_scalar(yi[:], varp.bitcast(I32), 1, None, OP.logical_shift_right)
    nc.vector.tensor_tensor(yi[:], magic[:], yi[:], OP.subtract)
    y = yi.bitcast(F32)
    e = pool.tile(sh, F32, tag="nr_e", bufs=2)
    h = pool.tile(sh, F32, tag="nr_h", bufs=2)
    for _ in range(3):
        nc.vector.tensor_tensor(e[:], y[:], y[:], OP.mult)
        nc.vector.tensor_tensor(e[:], e[:], varp[:], OP.mult)
        nc.vector.tensor_scalar(h[:], e[:], -0.5, 1.5, OP.mult, OP.add)
        nc.vector.tensor_tensor(y[:], y[:], h[:], OP.mult)
    nc.vector.tensor_copy(out[:], y[:])


def _layernorm(nc, st_pool, z_sb, y_sb):
    """Natural-layout LN: stats from bn_stats/aggr, NR rsqrt, gpsimd apply.

    z_sb: [128, NT, EMB] fp32. Writes normalized (no affine) y_sb (may be the
    same tile as z_sb for an in-place apply).
    """
    stats6 = st_pool.tile([128, NT, 6], F32, tag="st6", bufs=2)
    agg = st_pool.tile([128, NT, 2], F32, tag="agg", bufs=2)
    for tb in range(NT):
        nc.vector.bn_stats(stats6[:, tb], z_sb[:, tb])
        nc.vector.bn_aggr(agg[:, tb], stats6[:, tb])
    mean = agg[:, :, 0]
    varp = st_pool.tile([128, NT], F32, tag="varp", bufs=2)
    rstd = st_pool.tile([128, NT], F32, tag="rstd", bufs=2)
    mrstd = st_pool.tile([128, NT], F32, tag="mrstd", bufs=2)
    nc.vector.tensor_scalar(varp[:], agg[:, :, 1], 1.0, LN_EPS, OP.mult, OP.add)
    _nr_rsqrt(nc, st_pool, rstd, varp)
    nc.vector.tensor_tensor(mrstd[:], mean, rstd[:], OP.mult)
    for tb in range(NT):
        eng = nc.vector if os.environ.get("KDBG_NO_GPSIMD") else nc.gpsimd
        eng.tensor_scalar(
            y_sb[:, tb], z_sb[:, tb],
            rstd[:, tb:tb + 1], mrstd[:, tb:tb + 1],
            OP.mult, OP.subtract,
        )


def build_decoder(apply_g1be1: bool, apply_g2be2: bool, apply_b2: bool,
                  apply_b1: bool):
    """Build the per-core Bass program (B_LOC batches, full T each)."""
    nc = bacc.Bacc(None, target_bir_lowering=False)

    xp_d = nc.dram_tensor("xp", [B_LOC, T, EMB], F32, kind="ExternalInput")
    xt8_d = nc.dram_tensor("xt8", [B_LOC, EMB, T], F8, kind="ExternalInput")
    wq8_d = nc.dram_tensor("wq8", [EMB, 128], F8, kind="ExternalInput")
    wk8_d = nc.dram_tensor("wk8", [EMB, 128], F8, kind="ExternalInput")
    wv8_d = nc.dram_tensor("wv8", [EMB, DK], F8, kind="ExternalInput")
    wpf_d = nc.dram_tensor("wpf", [DK + 2, EMB + 2], BF16, kind="ExternalInput")
    w18_d = nc.dram_tensor("w18", [EMB, HID], F8, kind="ExternalInput")
    b1_d = nc.dram_tensor("b1", [128, 8], F32, kind="ExternalInput")
    w28_d = nc.dram_tensor("w28", [HID, EMB], F8, kind="ExternalInput")
    aff_d = nc.dram_tensor("aff", [1, 5, EMB], F32, kind="ExternalInput")
    idents_d = nc.dram_tensor("idents", [2, 128, 128], F32, kind="ExternalInput")
    # aff rows: b2, g1, be1, g2, be2
    out_d = nc.dram_tensor("out", [B_LOC, T, EMB], F32, kind="ExternalOutput")

    need_bcast = apply_g1be1 or apply_g2be2
    use_rt = not os.environ.get("KDBG_NO_RT")

    with TileContext(nc) as tc:
        with (
            tc.tile_pool(name="wpool", bufs=1) as wp,
            tc.tile_pool(name="xpool", bufs=2) as xq,
            tc.tile_pool(name="qkpool", bufs=1) as qk_pool,
            tc.tile_pool(name="atpool", bufs=4) as at_pool,
            tc.tile_pool(name="bigpool", bufs=1) as big_pool,
            tc.tile_pool(name="hpool", bufs=2) as h_pool,
            tc.tile_pool(name="stats", bufs=2) as st_pool,
        ):
            # ---------- weights / constants ----------
            ident = wp.tile([128, 128], F32R)
            nc.sync.dma_start(ident[:], idents_d[0].bitcast(F32R))
            ident16 = wp.tile([128, 128], F32R)
            nc.sync.dma_start(ident16[:], idents_d[1].bitcast(F32R))
            wq8_sb = wp.tile([128, 2, 128], F8)
            nc.sync.dma_start(wq8_sb[:], wq8_d.rearrange("(eb p) m -> p eb m", p=128))
            wk8_sb = wp.tile([128, 2, 128], F8)
            nc.sync.dma_start(wk8_sb[:], wk8_d.rearrange("(eb p) m -> p eb m", p=128))
            wv8_sb = wp.tile([128, 2, DK], F8)
            nc.sync.dma_start(wv8_sb[:], wv8_d.rearrange("(eb p) m -> p eb m", p=128))
            wpf_sb = wp.tile([DK + 2, EMB + 2], BF16)
            nc.sync.dma_start(wpf_sb[:], wpf_d[:])
            w18_sb = wp.tile([128, 2, HID], F8)
            nc.sync.dma_start(w18_sb[:], w18_d.rearrange("(eb p) m -> p eb m", p=128))
            w28_sb = wp.tile([128, 8, EMB], F8)
            nc.sync.dma_start(w28_sb[:], w28_d.rearrange("(hb p) m -> p hb m", p=128))
            if apply_b1:
                b1_sb = wp.tile([128, 8], F32)
                nc.sync.dma_start(b1_sb[:], b1_d[:])
            if need_bcast or apply_b2:
                ones1_sb = wp.tile([1, 128], F32R)
                nc.vector.memset(ones1_sb[:].bitcast(I32), 0x3F800000)
                aff_sb = wp.tile([1, 5, EMB], F32R)
                nc.sync.dma_start(aff_sb[:], aff_d[:].bitcast(F32R))
            if need_bcast:
                with tc.tile_pool(name="psbc", bufs=1, space="PSUM") as psbc:
                    ps_b = psbc.tile([128, 4, EMB], F32, tag="bc")
                    for i in range(4):
                        nc.tensor.matmul(
                            ps_b[:, i], ones1_sb[:], aff_sb[:, 1 + i],
                            start=True, stop=True,
                        )
                    affb_sb = wp.tile([128, 4, EMB], F32)
                    nc.vector.tensor_copy(affb_sb[:], ps_b[:])

            def attention(b):
                # ---------- loads ----------
                xt8_sb = xq.tile([128, 2, T], F8, tag="xt", bufs=2)
                nc.sync.dma_start(
                    xt8_sb[:], xt8_d[b].rearrange("(eb p) t -> p eb t", p=128)
                )
                xp_sb = xq.tile([128, NT, EMB], F32, tag="xp", bufs=2)
                nc.sync.dma_start(
                    xp_sb[:], xp_d[b].rearrange("(nt p) e -> p nt e", p=128)
                )

                qT_sb = qk_pool.tile([128, T], BF16, tag="qT", bufs=2)
                kT_sb = qk_pool.tile([128, T], BF16, tag="kT", bufs=2)
                v2 = qk_pool.tile([128, 8, 2, VPAD], F8, tag="v", bufs=2)
                attn_sb = qk_pool.tile([DK + 2, T], BF16, tag="attn", bufs=2)

                with tc.tile_pool(name="psatt", bufs=1, space="PSUM") as psatt:
                    # q,k projections (x4 replicated rows), DoubleRow over
                    # the 2 eb k-tiles, per 512-chunk
                    for c in range(NCH):
                        for w8, dst in ((wq8_sb, qT_sb), (wk8_sb, kT_sb)):
                            ps_qk = psatt.tile([128, 512], F32, tag="qk", bufs=2)
                            nc.tensor.matmul(
                                ps_qk[:],
                                w8[:, 0:2, :],
                                xt8_sb[:, 0:2, c * 512:(c + 1) * 512],
                                start=True, stop=True,
                                perf_mode=DR,
                            )
                            nc.vector.tensor_copy(
                                dst[:, c * 512:(c + 1) * 512], ps_qk[:]
                            )

                    # v projection (natural [s, dk]) + ones column, fp8
                    ps_v = psatt.tile([128, 8, 2, DK], F32, tag="v", bufs=1)
                    for tb in range(NT):
                        for eb in range(2):
                            nc.tensor.matmul(
                                ps_v[:, tb // 2, tb % 2],
                                xt8_sb[:, eb, tb * 128:(tb + 1) * 128],
                                wv8_sb[:, eb],
                                start=(eb == 0), stop=(eb == 1),
                            )
                    nc.vector.tensor_copy(v2[:, :, :, 0:DK], ps_v[:])
                    nc.vector.memset(v2[:, :, :, DK:DK + 1], 1.0)
                    nc.vector.memset(v2[:, :, :, DK + 1:VPAD], 0.0)

                    # attention: scoresT -> exp -> (mask) -> paired DR accum
                    for j in range(NCH):
                        t0 = j * 512
                        ps_at = psatt.tile([DK + 2, 512], F32, tag="at", bufs=1)
                        n_pair = 2 * j + 2
                        for p in range(n_pair):
                            a2 = at_pool.tile([128, 2, 512], F8, tag="aT", bufs=4)
                            lo = [0, 0]
                            for m in range(2):
                                sb = 2 * p + m
                                lo[m] = max(0, sb * 128 - t0)
                                grp = (sb % 4) * DK if use_rt else 0
                                ps_sc = psatt.tile([128, 512], F32, tag="sc", bufs=4)
                                nc.tensor.matmul(
                                    ps_sc[:, lo[m]:512],
                                    kT_sb[grp:grp + DK, sb * 128:(sb + 1) * 128],
                                    qT_sb[grp:grp + DK, t0 + lo[m]:t0 + 512],
                                    start=True, stop=True,
                                    tile_position=(grp, 0) if use_rt else None,
                                )
                                nc.scalar.activation(
                                    a2[:, m, lo[m]:512], ps_sc[:, lo[m]:512],
                                    AF.Exp, scale=ESCALE,
                                )
                                if sb * 128 >= t0:  # diagonal block: causal mask
                                    nc.gpsimd.affine_select(
                                        out=a2[:, m, lo[m]:lo[m] + 128],
                                        in_=a2[:, m, lo[m]:lo[m] + 128],
                                        compare_op=OP.is_ge,
                                        fill=0.0,
                                        base=0,
                                        pattern=[[1, 128]],
                                        channel_multiplier=-1,
                                    )
                            if lo[1] > lo[0]:  # zero the odd member's gap
                                nc.gpsimd.memset(a2[:, 1, lo[0]:lo[1]], 0.0)
                            nc.tensor.matmul(
                                ps_at[:, lo[0]:512],
                                v2[:, p, 0:2, 0:DK + 2],
                                a2[:, 0:2, lo[0]:512],
                                start=(p == 0), stop=(p == n_pair - 1),
                                perf_mode=DR,
                            )
                        nc.vector.tensor_copy(attn_sb[:, t0:t0 + 512], ps_at[:])

                # ---------- mh + z1 + LN1 + transpose ----------
                z1_sb = big_pool.tile([128, NT, EMB], F32, tag="zres", bufs=2)
                y1_sb = big_pool.tile([128, NT, EMB], F32R, tag="y1", bufs=2)
                recip = st_pool.tile([128, NT], F32, tag="recip", bufs=2)
                y1T8 = big_pool.tile([128, 2, T], F8, tag="y1T8", bufs=2)
                with (
                    tc.tile_pool(name="psmh", bufs=1, space="PSUM") as psmh,
                    tc.tile_pool(name="pstr", bufs=1, space="PSUM") as pstr,
                ):
                    for g in range(NT // 2):
                        ps_mh = psmh.tile([128, 2, 512], F32, tag="mh", bufs=2)
                        for i in range(2):
                            tb = 2 * g + i
                            nc.tensor.matmul(
                                ps_mh[:, i, 0:EMB + 2],
                                attn_sb[:, tb * 128:(tb + 1) * 128],
                                wpf_sb[:],
                                start=True, stop=True,
                            )
                        nc.vector.reciprocal(
                            recip[:, 2 * g:2 * g + 2], ps_mh[:, :, EMB]
                        )
                        for i in range(2):
                            tb = 2 * g + i
                            nc.vector.scalar_tensor_tensor(
                                out=z1_sb[:, tb],
                                in0=ps_mh[:, i, 0:EMB],
                                scalar=recip[:, tb:tb + 1],
                                in1=xp_sb[:, tb],
                                op0=OP.mult,
                                op1=OP.add,
                            )

                    _layernorm(nc, st_pool, z1_sb, y1_sb)
                    if apply_g1be1:
                        nc.vector.tensor_tensor(
                            y1_sb[:], y1_sb[:],
                            affb_sb[:, 0:1, :].to_broadcast([128, NT, EMB]),
                            OP.mult,
                        )
                        nc.vector.tensor_tensor(
                            y1_sb[:], y1_sb[:],
                            affb_sb[:, 1:2, :].to_broadcast([128, NT, EMB]),
                            OP.add,
                        )

                    for eb in range(2):
                        for half in range(2):
                            ps_tr = pstr.tile([128, 1024], F32R, tag="tr", bufs=2)
                            for q in range(8):
                                tb = half * 8 + q
                                nc.tensor.transpose(
                                    ps_tr[:, q * 128:(q + 1) * 128],
                                    y1_sb[:, tb, eb * 128:(eb + 1) * 128],
                                    ident[:],
                                )
                            nc.vector.tensor_scalar(
                                y1T8[:, eb, half * 1024:(half + 1) * 1024],
                                ps_tr[:].bitcast(F32), SY, None, OP.mult,
                            )
                return {"y1": y1_sb, "y1T8": y1T8, "b": b}

            def ffn(ctx):
                y1_sb, y1T8, b = ctx["y1"], ctx["y1T8"], ctx["b"]
                z2_sb = big_pool.tile([128, NT, EMB], F32, tag="zres", bufs=2)
                with tc.tile_pool(name="psffn", bufs=1, space="PSUM") as psffn:
                    for qtr in range(4):
                        hT8 = h_pool.tile([128, 8, 512], F8, tag="hT8", bufs=2)
                        for hp in range(4):
                            ps_h = psffn.tile([128, 2, 512], F32, tag="h", bufs=2)
                            for m in range(2):
                                h = 2 * hp + m
                                nc.tensor.matmul(
                                    ps_h[:, m],
                                    w18_sb[:, 0:2, h * 128:(h + 1) * 128],
                                    y1T8[:, 0:2, qtr * 512:(qtr + 1) * 512],
                                    start=True, stop=True,
                                    perf_mode=DR,
                                )
                            if apply_b1:
                                for m in range(2):
                                    h = 2 * hp + m
                                    nc.scalar.activation(
                                        hT8[:, h], ps_h[:, m], AF.Gelu,
                                        bias=b1_sb[:, h:h + 1], scale=GSCALE,
                                    )
                            else:
                                nc.scalar.activation(
                                    hT8[:, 2 * hp:2 * hp + 2], ps_h[:, 0:2],
                                    AF.Gelu, scale=GSCALE,
                                )
                        ps_ff = psffn.tile([128, 4, EMB], F32, tag="ff", bufs=2)
                        for i in range(4):
                            tb = qtr * 4 + i
                            if apply_b2:
                                nc.tensor.matmul(
                                    ps_ff[:, i], ones1_sb[:], aff_sb[:, 0],
                                    start=True, stop=False,
                                )
                            for kp in range(4):
                                nc.tensor.matmul(
                                    ps_ff[:, i],
                                    hT8[:, 2 * kp:2 * kp + 2, i * 128:(i + 1) * 128],
                                    w28_sb[:, 2 * kp:2 * kp + 2, :],
                                    start=(kp == 0 and not apply_b2),
                                    stop=False,
                                    perf_mode=DR,
                                )
                            # residual: accumulate SW2 * y1 via identity matmul
                            nc.tensor.matmul(
                                ps_ff[:, i],
                                ident16[:],
                                y1_sb[:, tb],
                                start=False, stop=True,
                            )
                        nc.vector.tensor_scalar(
                            z2_sb[:, qtr * 4:qtr * 4 + 4], ps_ff[:],
                            1.0 / SW2, None, OP.mult,
                        )

                    _layernorm(nc, st_pool, z2_sb, z2_sb)  # in-place apply
                    if apply_g2be2:
                        nc.vector.tensor_tensor(
                            z2_sb[:], z2_sb[:],
                            affb_sb[:, 2:3, :].to_broadcast([128, NT, EMB]),
                            OP.mult,
                        )
                        nc.vector.tensor_tensor(
                            z2_sb[:], z2_sb[:],
                            affb_sb[:, 3:4, :].to_broadcast([128, NT, EMB]),
                            OP.add,
                        )
                    nc.sync.dma_start(
                        out_d[b].rearrange("(nt p) e -> p nt e", p=128), z2_sb[:]
                    )

            def _emit_iter():
                ctxs = [attention(b) for b in range(B_LOC)]
                for ctx in ctxs:
                    ffn(ctx)

            LOOP_N = int(os.environ.get("KDBG_LOOP", "0"))
            if LOOP_N:
                with tc.For_i(0, LOOP_N, 1):
                    _emit_iter()
            else:
                _emit_iter()

    nc.compile()
    return nc


_CACHE = {}


def _get_nc(flags):
    if flags not in _CACHE:
        _CACHE[flags] = build_decoder(*flags)
    return _CACHE[flags]


def make_in_maps(x, Wq, Wk, Wv, Wp, bp, W1, b1, W2, b2, g1, be1, g2, be2):
    """Host-side preprocessing; returns per-core input maps + build flags."""
    f = np.asarray
    F8NP = ml_dtypes.float8_e4m3
    BFNP = ml_dtypes.bfloat16
    x = f(x, np.float32)
    wq8 = np.tile(f(Wq, np.float32) * SQ, (1, 4)).astype(F8NP)
    wk8 = np.tile(f(Wk, np.float32) * SK, (1, 4)).astype(F8NP)
    wv8 = (f(Wv, np.float32) * SV).astype(F8NP)
    wpf = np.zeros((DK + 2, EMB + 2), np.float32)
    wpf[0:DK, 0:EMB] = f(Wp, np.float32).reshape(EMB // DK, DK, EMB).sum(axis=0) / SV
    wpf[DK, EMB] = 1.0
    xp = (x + f(bp, np.float32)[None, None, :]).astype(np.float32)
    xt8 = np.ascontiguousarray(
        np.transpose(x.astype(F8NP), (0, 2, 1))
    )
    w18 = (f(W1, np.float32) * SW1).astype(F8NP)
    w28 = (f(W2, np.float32) * SW2).astype(F8NP)
    b1m = np.ascontiguousarray(f(b1, np.float32).reshape(8, 128).T)
    aff = np.stack(
        [f(b2), f(g1), f(be1), f(g2), f(be2)]
    ).astype(np.float32)[None]

    flags = (
        not (np.all(f(g1) == 1.0) and np.all(f(be1) == 0.0)),
        not (np.all(f(g2) == 1.0) and np.all(f(be2) == 0.0)),
        bool(np.any(f(b2) != 0.0)),
        bool(np.any(f(b1) != 0.0)),
    )
    idents = np.stack([np.eye(128, dtype=np.float32),
                       np.eye(128, dtype=np.float32) * SW2])
    shared = {
        "idents": idents,
        "wq8": wq8,
        "wk8": wk8,
        "wv8": wv8,
        "wpf": wpf.astype(BFNP),
        "w18": w18,
        "b1": b1m,
        "w28": w28,
        "aff": aff,
    }
    in_maps = []
    for c in range(N_CORES):
        sl = slice(c * B_LOC, (c + 1) * B_LOC)
        in_maps.append({"xp": xp[sl], "xt8": xt8[sl], **shared})
    return in_maps, flags


def kernel(**inputs) -> np.ndarray:
    in_maps, flags = make_in_maps(**inputs)
    nc = _get_nc(flags)
    res = run_bass_kernel_spmd(nc, in_maps, core_ids=list(range(N_CORES)))
    return np.concatenate([r["out"] for r in res.results], axis=0)


# revision 7
# speedup vs baseline: 1.0649x; 1.0649x over previous
"""Trainium2 Bass kernel for nn_Decoder_58377195487266.

Single-layer decoder: shared-head causal attention (d_k=32) + FFN(256->1024->256)
with two LayerNorms. B=16, T=2048, EMB=256.

Sharding: pure data-parallel over batch. 8 cores x 2 batches each, weights
replicated, no collectives.

Numerics/layout strategy (validated vs reference in numpy, absmax_rel ~6e-3):
  - fp8e4m3 (TRN FP8_EXP4) inputs for all heavy matmuls, with host-side scale
    folding (weights pre-scaled so values stay in e4m3's normal range; the
    inverse scales fold into free activation-scale slots downstream).
  - DoubleRow fp8 matmuls (2 k-tiles of 128 packed per instruction) for the
    q/k projections, the attention accumulate, and both FFN matmuls.
  - Scores matmul in bf16 (K=32) with 4-way PE row tiling (tile_position).
  - exp via ACT with the combined descale folded into the activation scale.
  - Softmax denominator via ones-column in v (paired/fp8), recip on DVE.
  - FFN residual (z2 = y1 + ff) accumulated into PSUM by a 16*I identity
    matmul (f32r), so the DVE pass is a cheap single-src tensor_scalar.
  - Batch phases ordered att(0),att(1),ffn(0),ffn(1) so the ACT table set
    switches exp<->gelu only twice per iteration.
"""

import math
import os

import numpy as np
import ml_dtypes

import concourse.bass as bass  # noqa: F401
import concourse.mybir as mybir
from concourse import bacc
from concourse.bass_utils import run_bass_kernel_spmd
from concourse.tile import TileContext

F32 = mybir.dt.float32
F32R = mybir.dt.float32r
BF16 = mybir.dt.bfloat16
F8 = mybir.dt.float8e4
I32 = mybir.dt.int32
AF = mybir.ActivationFunctionType
OP = mybir.AluOpType
DR = mybir.MatmulPerfMode.DoubleRow

B, T, EMB = 16, 2048, 256
DK = 32
HID = 4 * EMB
N_CORES = 8
B_LOC = B // N_CORES  # 2
NT = T // 128         # 16 t-blocks
NCH = T // 512        # 4 chunks
LN_EPS = 1e-5

# host-side scale folding (see module docstring)
SQ, SK, SV, SW1, SW2, SY = 64.0, 16.0, 16.0, 16.0, 16.0, 4.0
ESCALE = 1.0 / (SQ * SK * math.sqrt(DK))   # applied inside exp
GSCALE = 1.0 / (SY * SW1)                  # applied inside gelu
VPAD = 48                                  # v pair stride (DoubleRow needs %16==0)

RSQRT_MAGIC = 0x5F3759DF


def _nr_rsqrt(nc, pool, out, varp):
    """out = 1/sqrt(varp) via bit-trick + 3 Newton iterations, all on DVE."""
    sh = list(varp.shape)
    yi = pool.tile(sh, I32, tag="nr_i", bufs=2)
    magic = pool.tile(sh, I32, tag="nr_m", bufs=2)
    nc.vector.memset(magic[:], RSQRT_MAGIC)
    nc.vector.tensor_scalar(yi[:], varp.bitcast(I32), 1, None, OP.logical_shift_right)
    nc.vector.tensor_tensor(yi[:], magic[:], yi[:], OP.subtract)
    y = yi.bitcast(F32)
    e = pool.tile(sh, F32, tag="nr_e", bufs=2)
    h = pool.tile(sh, F32, tag="nr_h", bufs=2)
    for _ in range(3):
        nc.vector.tensor_tensor(e[:], y[:], y[:], OP.mult)
        nc.vector.tensor_tensor(e[:], e[:], varp[:], OP.mult)
        nc.vector.tensor_scalar(h[:], e[:], -0.5, 1.5, OP.mult, OP.add)
        nc.vector.tensor_tensor(y[:], y[:], h[:], OP.mult)
    nc.vector.tensor_copy(out[:], y[:])


def _layernorm(nc, st_pool, z_sb, y_sb):
    """Natural-layout LN: stats from bn_stats/aggr, NR rsqrt, gpsimd apply.

    z_sb: [128, NT, EMB] fp32. Writes normalized (no affine) y_sb (may be the
    same tile as z_sb for an in-place apply).
    """
    stats6 = st_pool.tile([128, NT, 6], F32, tag="st6", bufs=2)
    agg = st_pool.tile([128, NT, 2], F32, tag="agg", bufs=2)
    for tb in range(NT):
        nc.vector.bn_stats(stats6[:, tb], z_sb[:, tb])
        nc.vector.bn_aggr(agg[:, tb], stats6[:, tb])
    mean = agg[:, :, 0]
    varp = st_pool.tile([128, NT], F32, tag="varp", bufs=2)
    rstd = st_pool.tile([128, NT], F32, tag="rstd", bufs=2)
    mrstd = st_pool.tile([128, NT], F32, tag="mrstd", bufs=2)
    nc.vector.tensor_scalar(varp[:], agg[:, :, 1], 1.0, LN_EPS, OP.mult, OP.add)
    _nr_rsqrt(nc, st_pool, rstd, varp)
    nc.vector.tensor_tensor(mrstd[:], mean, rstd[:], OP.mult)
    for tb in range(NT):
        eng = nc.vector if os.environ.get("KDBG_NO_GPSIMD") else nc.gpsimd
        eng.tensor_scalar(
            y_sb[:, tb], z_sb[:, tb],
            rstd[:, tb:tb + 1], mrstd[:, tb:tb + 1],
            OP.mult, OP.subtract,
        )


def build_decoder(apply_g1be1: bool, apply_g2be2: bool, apply_b2: bool,
                  apply_b1: bool):
    """Build the per-core Bass program (B_LOC batches, full T each)."""
    nc = bacc.Bacc(None, target_bir_lowering=False)

    xp_d = nc.dram_tensor("xp", [B_LOC, T, EMB], F32, kind="ExternalInput")
    xt8_d = nc.dram_tensor("xt8", [B_LOC, EMB, T], F8, kind="ExternalInput")
    wq8_d = nc.dram_tensor("wq8", [EMB, 128], F8, kind="ExternalInput")
    wk8_d = nc.dram_tensor("wk8", [EMB, 128], F8, kind="ExternalInput")
    wv8_d = nc.dram_tensor("wv8", [EMB, DK], F8, kind="ExternalInput")
    wpf_d = nc.dram_tensor("wpf", [DK + 2, EMB + 2], BF16, kind="ExternalInput")
    w18_d = nc.dram_tensor("w18", [EMB, HID], F8, kind="ExternalInput")
    b1_d = nc.dram_tensor("b1", [128, 8], F32, kind="ExternalInput")
    w28_d = nc.dram_tensor("w28", [HID, EMB], F8, kind="ExternalInput")
    aff_d = nc.dram_tensor("aff", [1, 5, EMB], F32, kind="ExternalInput")
    idents_d = nc.dram_tensor("idents", [2, 128, 128], F32, kind="ExternalInput")
    # aff rows: b2, g1, be1, g2, be2
    out_d = nc.dram_tensor("out", [B_LOC, T, EMB], F32, kind="ExternalOutput")

    need_bcast = apply_g1be1 or apply_g2be2
    use_rt = not os.environ.get("KDBG_NO_RT")

    with TileContext(nc) as tc:
        with (
            tc.tile_pool(name="wpool", bufs=1) as wp,
            tc.tile_pool(name="xpool", bufs=2) as xq,
            tc.tile_pool(name="qkpool", bufs=1) as qk_pool,
            tc.tile_pool(name="atpool", bufs=4) as at_pool,
            tc.tile_pool(name="bigpool", bufs=1) as big_pool,
            tc.tile_pool(name="hpool", bufs=2) as h_pool,
            tc.tile_pool(name="stats", bufs=2) as st_pool,
        ):
            # ---------- weights / constants ----------
            ident = wp.tile([128, 128], F32)
            nc.sync.dma_start(ident[:], idents_d[0])
            wq8_sb = wp.tile([128, 2, 128], F8)
            nc.sync.dma_start(wq8_sb[:], wq8_d.rearrange("(eb p) m -> p eb m", p=128))
            wk8_sb = wp.tile([128, 2, 128], F8)
            nc.sync.dma_start(wk8_sb[:], wk8_d.rearrange("(eb p) m -> p eb m", p=128))
            wv8_sb = wp.tile([128, 2, DK], F8)
            nc.sync.dma_start(wv8_sb[:], wv8_d.rearrange("(eb p) m -> p eb m", p=128))
            wpf_sb = wp.tile([DK + 2, EMB + 2], BF16)
            nc.sync.dma_start(wpf_sb[:], wpf_d[:])
            w18_sb = wp.tile([128, 2, HID], F8)
            nc.sync.dma_start(w18_sb[:], w18_d.rearrange("(eb p) m -> p eb m", p=128))
            w28_sb = wp.tile([128, 8, EMB], F8)
            nc.sync.dma_start(w28_sb[:], w28_d.rearrange("(hb p) m -> p hb m", p=128))
            if apply_b1:
                b1_sb = wp.tile([128, 8], F32)
                nc.sync.dma_start(b1_sb[:], b1_d[:])
            if need_bcast or apply_b2:
                ones1_sb = wp.tile([1, 128], F32R)
                nc.vector.memset(ones1_sb[:].bitcast(I32), 0x3F800000)
                aff_sb = wp.tile([1, 5, EMB], F32R)
                nc.sync.dma_start(aff_sb[:], aff_d[:].bitcast(F32R))
            if need_bcast:
                with tc.tile_pool(name="psbc", bufs=1, space="PSUM") as psbc:
                    ps_b = psbc.tile([128, 4, EMB], F32, tag="bc")
                    for i in range(4):
                        nc.tensor.matmul(
                            ps_b[:, i], ones1_sb[:], aff_sb[:, 1 + i],
                            start=True, stop=True,
                        )
                    affb_sb = wp.tile([128, 4, EMB], F32)
                    nc.vector.tensor_copy(affb_sb[:], ps_b[:])

            def attention(b):
                # ---------- loads ----------
                xt8_sb = xq.tile([128, 2, T], F8, tag="xt", bufs=2)
                nc.sync.dma_start(
                    xt8_sb[:], xt8_d[b].rearrange("(eb p) t -> p eb t", p=128)
                )
                xp_sb = xq.tile([128, NT, EMB], F32, tag="xp", bufs=2)
                nc.sync.dma_start(
                    xp_sb[:], xp_d[b].rearrange("(nt p) e -> p nt e", p=128)
                )

                qT_sb = qk_pool.tile([128, T], BF16, tag="qT", bufs=2)
                kT_sb = qk_pool.tile([128, T], BF16, tag="kT", bufs=2)
                v_ext = qk_pool.tile([128, NT, DK + 2], BF16, tag="v", bufs=2)
                attn_sb = qk_pool.tile([DK + 2, T], BF16, tag="attn", bufs=2)

                with tc.tile_pool(name="psatt", bufs=1, space="PSUM") as psatt:
                    # q,k projections (x4 replicated rows), DoubleRow over
                    # the 2 eb k-tiles, per 512-chunk
                    for c in range(NCH):
                        for w8, dst in ((wq8_sb, qT_sb), (wk8_sb, kT_sb)):
                            ps_qk = psatt.tile([128, 512], F32, tag="qk", bufs=2)
                            nc.tensor.matmul(
                                ps_qk[:],
                                w8[:, 0:2, :],
                                xt8_sb[:, 0:2, c * 512:(c + 1) * 512],
                                start=True, stop=True,
                                perf_mode=DR,
                            )
                            nc.vector.tensor_copy(
                                dst[:, c * 512:(c + 1) * 512], ps_qk[:]
                            )

                    # v projection (natural [s, dk]) + ones column, fp8
                    ps_v = psatt.tile([128, NT, DK], F32, tag="v", bufs=1)
                    for tb in range(NT):
                        for eb in range(2):
                            nc.tensor.matmul(
                                ps_v[:, tb],
                                xt8_sb[:, eb, tb * 128:(tb + 1) * 128],
                                wv8_sb[:, eb],
                                start=(eb == 0), stop=(eb == 1),
                            )
                    nc.vector.tensor_copy(v_ext[:, :, 0:DK], ps_v[:])
                    nc.vector.memset(v_ext[:, :, DK:DK + 1], 1.0)
                    nc.vector.memset(v_ext[:, :, DK + 1:DK + 2], 0.0)

                    # attention: scoresT -> exp -> (mask) -> paired DR accum
                    for j in range(NCH):
                        t0 = j * 512
                        ps_at = psatt.tile([DK + 2, 512], F32, tag="at", bufs=1)
                        n_sb = 4 * j + 4
                        for sb in range(n_sb):
                            lo = max(0, sb * 128 - t0)
                            grp = (sb % 4) * DK if use_rt else 0
                            ps_sc = psatt.tile([128, 512], F32, tag="sc", bufs=4)
                            nc.tensor.matmul(
                                ps_sc[:, lo:512],
                                kT_sb[grp:grp + DK, sb * 128:(sb + 1) * 128],
                                qT_sb[grp:grp + DK, t0 + lo:t0 + 512],
                                start=True, stop=True,
                                tile_position=(grp, 0) if use_rt else None,
                            )
                            a_t = at_pool.tile([128, 512], BF16, tag="aT", bufs=4)
                            nc.scalar.activation(
                                a_t[:, lo:512], ps_sc[:, lo:512],
                                AF.Exp, scale=ESCALE,
                            )
                            if sb * 128 >= t0:  # diagonal block: causal mask
                                nc.gpsimd.affine_select(
                                    out=a_t[:, lo:lo + 128],
                                    in_=a_t[:, lo:lo + 128],
                                    compare_op=OP.is_ge,
                                    fill=0.0,
                                    base=0,
                                    pattern=[[1, 128]],
                                    channel_multiplier=-1,
                                )
                            nc.tensor.matmul(
                                ps_at[:, lo:512],
                                v_ext[:, sb, :],
                                a_t[:, lo:512],
                                start=(sb == 0), stop=(sb == n_sb - 1),
                            )
                        nc.vector.tensor_copy(attn_sb[:, t0:t0 + 512], ps_at[:])

                # ---------- mh + z1 + LN1 + transpose ----------
                z1_sb = big_pool.tile([128, NT, EMB], F32, tag="zres", bufs=2)
                y1_sb = big_pool.tile([128, NT, EMB], F32, tag="y1", bufs=2)
                recip = st_pool.tile([128, NT], F32, tag="recip", bufs=2)
                y1T8 = big_pool.tile([128, 2, T], F8, tag="y1T8", bufs=2)
                with (
                    tc.tile_pool(name="psmh", bufs=1, space="PSUM") as psmh,
                    tc.tile_pool(name="pstr", bufs=1, space="PSUM") as pstr,
                ):
                    for g in range(NT // 2):
                        ps_mh = psmh.tile([128, 2, 512], F32, tag="mh", bufs=2)
                        for i in range(2):
                            tb = 2 * g + i
                            nc.tensor.matmul(
                                ps_mh[:, i, 0:EMB + 2],
                                attn_sb[:, tb * 128:(tb + 1) * 128],
                                wpf_sb[:],
                                start=True, stop=True,
                            )
                        nc.vector.reciprocal(
                            recip[:, 2 * g:2 * g + 2], ps_mh[:, :, EMB]
                        )
                        for i in range(2):
                            tb = 2 * g + i
                            nc.vector.scalar_tensor_tensor(
                                out=z1_sb[:, tb],
                                in0=ps_mh[:, i, 0:EMB],
                                scalar=recip[:, tb:tb + 1],
                                in1=xp_sb[:, tb],
                                op0=OP.mult,
                                op1=OP.add,
                            )

                    _layernorm(nc, st_pool, z1_sb, y1_sb)
                    if apply_g1be1:
                        nc.vector.tensor_tensor(
                            y1_sb[:], y1_sb[:],
                            affb_sb[:, 0:1, :].to_broadcast([128, NT, EMB]),
                            OP.mult,
                        )
                        nc.vector.tensor_tensor(
                            y1_sb[:], y1_sb[:],
                            affb_sb[:, 1:2, :].to_broadcast([128, NT, EMB]),
                            OP.add,
                        )

                    for eb in range(2):
                        for half in range(2):
                            ps_tr = pstr.tile([128, 1024], F32, tag="tr", bufs=2)
                            for q in range(8):
                                tb = half * 8 + q
                                nc.tensor.transpose(
                                    ps_tr[:, q * 128:(q + 1) * 128],
                                    y1_sb[:, tb, eb * 128:(eb + 1) * 128],
                                    ident[:],
                                )
                            nc.vector.tensor_scalar(
                                y1T8[:, eb, half * 1024:(half + 1) * 1024],
                                ps_tr[:], SY, None, OP.mult,
                            )
                return {"y1": y1_sb, "y1T8": y1T8, "b": b}

            def ffn(ctx):
                y1_sb, y1T8, b = ctx["y1"], ctx["y1T8"], ctx["b"]
                z2_sb = big_pool.tile([128, NT, EMB], F32, tag="zres", bufs=2)
                with tc.tile_pool(name="psffn", bufs=1, space="PSUM") as psffn:
                    for qtr in range(4):
                        hT8 = h_pool.tile([128, 8, 512], F8, tag="hT8", bufs=2)
                        for hp in range(4):
                            ps_h = psffn.tile([128, 2, 512], F32, tag="h", bufs=2)
                            for m in range(2):
                                h = 2 * hp + m
                                nc.tensor.matmul(
                                    ps_h[:, m],
                                    w18_sb[:, 0:2, h * 128:(h + 1) * 128],
                                    y1T8[:, 0:2, qtr * 512:(qtr + 1) * 512],
                                    start=True, stop=True,
                                    perf_mode=DR,
                                )
                            if apply_b1:
                                for m in range(2):
                                    h = 2 * hp + m
                                    nc.scalar.activation(
                                        hT8[:, h], ps_h[:, m], AF.Gelu,
                                        bias=b1_sb[:, h:h + 1], scale=GSCALE,
                                    )
                            else:
                                nc.scalar.activation(
                                    hT8[:, 2 * hp:2 * hp + 2], ps_h[:, 0:2],
                                    AF.Gelu, scale=GSCALE,
                                )
                        ps_ff = psffn.tile([128, 4, EMB], F32, tag="ff", bufs=2)
                        for i in range(4):
                            tb = qtr * 4 + i
                            if apply_b2:
                                nc.tensor.matmul(
                                    ps_ff[:, i], ones1_sb[:], aff_sb[:, 0],
                                    start=True, stop=False,
                                )
                            for kp in range(4):
                                nc.tensor.matmul(
                                    ps_ff[:, i],
                                    hT8[:, 2 * kp:2 * kp + 2, i * 128:(i + 1) * 128],
                                    w28_sb[:, 2 * kp:2 * kp + 2, :],
                                    start=(kp == 0 and not apply_b2),
                                    stop=(kp == 3),
                                    perf_mode=DR,
                                )
                        nc.vector.scalar_tensor_tensor(
                            out=z2_sb[:, qtr * 4:qtr * 4 + 4],
                            in0=ps_ff[:],
                            scalar=1.0 / SW2,
                            in1=y1_sb[:, qtr * 4:qtr * 4 + 4],
                            op0=OP.mult,
                            op1=OP.add,
                        )

                    _layernorm(nc, st_pool, z2_sb, z2_sb)  # in-place apply
                    if apply_g2be2:
                        nc.vector.tensor_tensor(
                            z2_sb[:], z2_sb[:],
                            affb_sb[:, 2:3, :].to_broadcast([128, NT, EMB]),
                            OP.mult,
                        )
                        nc.vector.tensor_tensor(
                            z2_sb[:], z2_sb[:],
                            affb_sb[:, 3:4, :].to_broadcast([128, NT, EMB]),
                            OP.add,
                        )
                    nc.sync.dma_start(
                        out_d[b].rearrange("(nt p) e -> p nt e", p=128), z2_sb[:]
                    )

            def _emit_iter():
                ctxs = [attention(b) for b in range(B_LOC)]
                for ctx in ctxs:
                    ffn(ctx)

            LOOP_N = int(os.environ.get("KDBG_LOOP", "0"))
            if LOOP_N:
                with tc.For_i(0, LOOP_N, 1):
                    _emit_iter()
            else:
                _emit_iter()

    nc.compile()
    return nc


_CACHE = {}


def _get_nc(flags):
    if flags not in _CACHE:
        _CACHE[flags] = build_decoder(*flags)
    return _CACHE[flags]


def make_in_maps(x, Wq, Wk, Wv, Wp, bp, W1, b1, W2, b2, g1, be1, g2, be2):
    """Host-side preprocessing; returns per-core input maps + build flags."""
    f = np.asarray
    F8NP = ml_dtypes.float8_e4m3
    BFNP = ml_dtypes.bfloat16
    x = f(x, np.float32)
    wq8 = np.tile(f(Wq, np.float32) * SQ, (1, 4)).astype(F8NP)
    wk8 = np.tile(f(Wk, np.float32) * SK, (1, 4)).astype(F8NP)
    wv8 = (f(Wv, np.float32) * SV).astype(F8NP)
    wpf = np.zeros((DK + 2, EMB + 2), np.float32)
    wpf[0:DK, 0:EMB] = f(Wp, np.float32).reshape(EMB // DK, DK, EMB).sum(axis=0) / SV
    wpf[DK, EMB] = 1.0
    xp = (x + f(bp, np.float32)[None, None, :]).astype(np.float32)
    xt8 = np.ascontiguousarray(
        np.transpose(x.astype(F8NP), (0, 2, 1))
    )
    w18 = (f(W1, np.float32) * SW1).astype(F8NP)
    w28 = (f(W2, np.float32) * SW2).astype(F8NP)
    b1m = np.ascontiguousarray(f(b1, np.float32).reshape(8, 128).T)
    aff = np.stack(
        [f(b2), f(g1), f(be1), f(g2), f(be2)]
    ).astype(np.float32)[None]

    flags = (
        not (np.all(f(g1) == 1.0) and np.all(f(be1) == 0.0)),
        not (np.all(f(g2) == 1.0) and np.all(f(be2) == 0.0)),
        bool(np.any(f(b2) != 0.0)),
        bool(np.any(f(b1) != 0.0)),
    )
    idents = np.stack([np.eye(128, dtype=np.float32),
                       np.eye(128, dtype=np.float32) * SW2])
    shared = {
        "idents": idents,
        "wq8": wq8,
        "wk8": wk8,
        "wv8": wv8,
        "wpf": wpf.astype(BFNP),
        "w18": w18,
        "b1": b1m,
        "w28": w28,
        "aff": aff,
    }
    in_maps = []
    for c in range(N_CORES):
        sl = slice(c * B_LOC, (c + 1) * B_LOC)
        in_maps.append({"xp": xp[sl], "xt8": xt8[sl], **shared})
    return in_maps, flags


def kernel(**inputs) -> np.ndarray:
    in_maps, flags = make_in_maps(**inputs)
    nc = _get_nc(flags)
    res = run_bass_kernel_spmd(nc, in_maps, core_ids=list(range(N_CORES)))
    return np.concatenate([r["out"] for r in res.results], axis=0)
